# revision 7
# baseline (speedup 1.0000x reference)
"""AdaptiveWingLoss on 8 TRN2 NeuronCores.

loss/14 = select(d < 0.5, log1p(d^p), ps*(2d-1) + lp)
  d  = |x - y|,  p = 2.1 - y,  z = ln2*(y - 2.1)
  e1 = exp(z) = 0.5^p,  lp = log1p(e1),  r = exp(-lp) = 1/(1+e1)
  ps = p * sigma(z) = p * (1 - r)
  log1p(d^p) = Ln(exp(-(y-2.1)*ln(d)) + 1)

All transcendentals come from the single {ln, exp} ACT table set (no table
switches). Batch dim sharded across 8 cores; each core returns a partial
sum; host adds the partials and scales by omega=14.
"""
import numpy as np

import concourse.bacc as bacc
import concourse.mybir as mybir
from concourse.tile import TileContext
from concourse.bass_utils import run_bass_kernel_spmd

N_CORES = 8
ROWS, COLS = 1024, 2048  # per-core shard, fp32 elements
NT = ROWS // 128
LN2 = float(np.log(2.0))

F32 = mybir.dt.float32
BF16 = mybir.dt.bfloat16
AF = mybir.ActivationFunctionType
ALU = mybir.AluOpType

_CACHE = {}


def _register_const(nc, value, dtype=F32):
    t = nc.alloc_sbuf_tensor(f"const-{dtype.name}-{value}", [128, 1], dtype)
    nc.gpsimd.memset(t.ap(), value)
    nc.const_aps.aps[(dtype, value)] = t.ap()


def _build():
    nc = bacc.Bacc(None, target_bir_lowering=False)
    _register_const(nc, -2.1 * LN2)
    _register_const(nc, 1e-37)
    nc.all_engine_barrier()
    x_ext = nc.declare_dram_parameter("x", [ROWS, COLS], F32, isOutput=False)
    y_ext = nc.declare_dram_parameter("y", [ROWS, COLS], F32, isOutput=False)
    out_ext = nc.declare_dram_parameter("out", [1, 1], F32, isOutput=True)

    with TileContext(nc) as tc:
        with (
            tc.tile_pool(name="io", bufs=2) as iop,
            tc.tile_pool(name="work", bufs=16) as wp,
            tc.tile_pool(name="mask", bufs=2) as mp,
            tc.tile_pool(name="accp", bufs=1) as accp,
        ):
            acc_all = accp.tile([128, NT], F32, tag="accall")

            for t in range(NT):
                xt = iop.tile([128, COLS], F32, tag="x")
                yt = iop.tile([128, COLS], F32, tag="y")
                nc.sync.dma_start(out=xt[:, :], in_=x_ext[t * 128:(t + 1) * 128, :])
                nc.sync.dma_start(out=yt[:, :], in_=y_ext[t * 128:(t + 1) * 128, :])

                def w(nm):
                    return wp.tile([128, COLS], F32, tag="w", name=f"{nm}_{t}")

                d0 = w("d0")
                nc.vector.tensor_tensor(d0[:, :], xt[:, :], yt[:, :], ALU.subtract)
                d = w("d")
                nc.scalar.activation(d[:, :], d0[:, :], AF.Abs)
                ld = w("ld")
                nc.scalar.activation(ld[:, :], d[:, :], AF.Ln, bias=1e-37)
                pm = w("pm")  # y - 2.1 = -p
                nc.vector.tensor_scalar(pm[:, :], yt[:, :], -2.1, None, ALU.add)
                up = w("up")  # -p*ln(d)
                nc.vector.tensor_tensor(up[:, :], pm[:, :], ld[:, :], ALU.mult)
                e2 = w("e2")  # d^p
                nc.scalar.activation(e2[:, :], up[:, :], AF.Exp, scale=-1.0)
                nl = w("nl")  # log1p(d^p)
                nc.scalar.activation(nl[:, :], e2[:, :], AF.Ln, bias=1.0)
                e1 = w("e1")  # 0.5^p
                nc.scalar.activation(e1[:, :], yt[:, :], AF.Exp, scale=LN2, bias=-2.1 * LN2)
                lp = w("lp")  # log1p(0.5^p)
                nc.scalar.activation(lp[:, :], e1[:, :], AF.Ln, bias=1.0)
                r = w("r")  # 1/(1+e1)
                nc.scalar.activation(r[:, :], lp[:, :], AF.Exp, scale=-1.0)
                sg = w("sg")  # sigma(z) = 1 - r
                nc.vector.tensor_scalar(sg[:, :], r[:, :], -1.0, 1.0, ALU.mult, ALU.add)
                nps = w("nps")  # -ps = (y-2.1)*sigma
                nc.vector.tensor_tensor(nps[:, :], pm[:, :], sg[:, :], ALU.mult)
                dm = w("dm")  # 1 - 2d
                nc.vector.tensor_scalar(dm[:, :], d[:, :], -2.0, 1.0, ALU.mult, ALU.add)
                lin0 = w("lin0")  # ps*(2d-1)
                nc.vector.tensor_tensor(lin0[:, :], nps[:, :], dm[:, :], ALU.mult)
                lin = w("lin")
                nc.vector.tensor_tensor(lin[:, :], lin0[:, :], lp[:, :], ALU.add)
                m = mp.tile([128, COLS], mybir.dt.uint8, tag="m", name=f"m_{t}")  # 1 where d >= 0.5
                nc.vector.tensor_scalar(m[:, :], d[:, :], 0.5, None, ALU.is_ge)
                # nl <- lin where d >= 0.5; nl is now loss/14
                nc.vector.copy_predicated(nl[:, :], m[:, :], lin[:, :])
                # free-dim sum into acc_all[:, t]
                junk = w("junk")
                nc.vector.tensor_scalar(
                    junk[:, :], nl[:, :], 1.0, 0.0, ALU.mult, ALU.add,
                    accum_out=acc_all[:, t:t + 1],
                )

            red = accp.tile([128, 1], F32, tag="red")
            nc.vector.tensor_reduce(red[:, :], acc_all[:, :], mybir.AxisListType.X, ALU.add)
            tot = accp.tile([1, 1], F32, tag="tot")
            nc.gpsimd.tensor_reduce(tot[:, :], red[:, :], mybir.AxisListType.C, ALU.add)
            nc.sync.dma_start(out=out_ext[:, :], in_=tot[:, :])

    nc.compile()
    return nc


def _get_nc():
    if "nc" not in _CACHE:
        _CACHE["nc"] = _build()
    return _CACHE["nc"]


def kernel(input, target):
    x = np.ascontiguousarray(input, dtype=np.float32).reshape(N_CORES, ROWS, COLS)
    y = np.ascontiguousarray(target, dtype=np.float32).reshape(N_CORES, ROWS, COLS)
    nc = _get_nc()
    in_maps = [{"x": x[i], "y": y[i]} for i in range(N_CORES)]
    res = run_bass_kernel_spmd(nc, in_maps, core_ids=list(range(N_CORES)))
    total = sum(float(res.results[i]["out"][0, 0]) for i in range(N_CORES))
    return np.float32(14.0 * total)


# revision 8
# speedup vs baseline: 1.2543x; 1.2543x over previous
"""AdaptiveWingLoss on 8 TRN2 NeuronCores.

Math (theta=0.5, eps=1, alpha=2.1, omega=14):
  d  = |x - y|,  p = 2.1 - y,  pm = y - 2.1,  z = ln2*pm
  nl  = log1p(d^p) = Ln(exp(p*ln d) + 1)
  lp  = log1p(e^z),  sigma = 1/(1+e^-z),  ps = p*sigma
  lin = ps*(2d-1) + lp = A'(y)*(d-0.5) + lp,  A' = 2*ps = -2*pm*sigma(z)
  loss/14 = select(d<0.5, nl, lin)

Key identity: d >= 0.5  <=>  nl >= lp (monotone), so the select dissolves:
  sum(loss)/14 = sum(nl) + sum(A' * relu(d-0.5)) + sum(min(lp - nl, 0))

sigma(z) and lp are approximated by quadratics in pm (max rel err ~1e-3,
end-to-end ~3e-6), evaluated inside fused custom DVE ops with accum=ADD.
All ACT work is in the single {ln, exp} table set (no table switches).
Batch dim sharded across 8 cores; each core emits per-partition partial
sums [128,1]; host adds and scales by 14.
"""
import numpy as np

import concourse.bacc as bacc
import concourse.mybir as mybir
import concourse.dve_ops as dops
from concourse.dve_spec import Spec, Src0, Src1, C0, C1, C2, Zero, lower, maxx, minn, relu, _has_src1
from concourse.dve_uop import DveOpSpec
from concourse.tile import TileContext
from concourse.bass_utils import run_bass_kernel_spmd

N_CORES = 8
ROWS, COLS = 1024, 2048  # per-core shard, fp32 elements
NT = ROWS // 128
LN2 = float(np.log(2.0))
D_EPS = 1e-6

# quadratic fits on pm = y - 2.1 in [-2.1, -1.1] (np.polyfit, deg 2)
# sigma(ln2*pm) ~ S0 + S1*pm + S2*pm^2 ; A' = -2*pm*sigma -> coefs *(-2)
QS = (-2 * 0.5118057, -2 * 0.20058717, -2 * 0.02234001)
# log1p(exp(ln2*pm)) ~ R0 + R1*pm + R2*pm^2
QR = (0.6760186, 0.31594962, 0.04476109)

F32 = mybir.dt.float32
BF16 = mybir.dt.bfloat16
AF = mybir.ActivationFunctionType
ALU = mybir.AluOpType

_CACHE = {}


def _make_dve_op(name, spec):
    """Register a custom DVE op at runtime (name -> free opcode row)."""
    existing = {op.name: op for op in dops.OPS}
    if name in existing:
        return existing[name]
    row = dops._CUSTOM_DVE_ROW_BASE + len(dops.OPS)
    tmp = DveOpSpec(name=name, opcode=row, uops=lower(spec, ver="v3"),
                    rd1_en=_has_src1(spec))
    op = dops.DveOp(name, spec, subdim=False, uops_sha={"v3": tmp.sha("v3")})
    dops.OPS.append(op)
    dops._SUB_OPCODE_FOR_NAME[name] = row
    dops.CUSTOM_DVE_SPECS[name] = spec
    return op


def _get_ops():
    if "ops" in _CACHE:
        return _CACHE["ops"]
    # ds = max(|x - y|, eps) - 0.5
    absdiff = _make_dve_op(
        "AWL_ABSDIFF_SHIFT",
        Spec(
            body=maxx(maxx(Src0 - Src1, Src1 - Src0), C1) - C0,
            reference=lambda in0, in1, s0, s1, imm2: (
                np.maximum(np.abs(in0.astype(np.float32) - in1), s1) - s0
            ),
        ),
    )
    # accum += relu(ds) * pm * (q0 + q1*pm + q2*pm^2); ds=Src0, pm=Src1
    lin_red = _make_dve_op(
        "AWL_LIN_REDUCE",
        Spec(
            body=relu(Src0) * (Src1 * ((C2 * Src1 + C1) * Src1 + C0)),
            accum=dops.add,
            accum_init=Zero,
            reference=lambda in0, in1, s0, s1, imm2: (
                lambda b: (b, b.reshape(b.shape[0], -1).sum(-1, keepdims=True))
            )(
                np.maximum(in0.astype(np.float32), 0)
                * (in1 * ((imm2 * in1 + s1) * in1 + s0))
            ),
        ),
    )
    # accum += min((r0 + r1*pm + r2*pm^2) - nl, 0); nl=Src0, pm=Src1
    lp_min_red = _make_dve_op(
        "AWL_LPMIN_REDUCE",
        Spec(
            body=minn(((C2 * Src1 + C1) * Src1 + C0) - Src0, Zero),
            accum=dops.add,
            accum_init=Zero,
            reference=lambda in0, in1, s0, s1, imm2: (
                lambda b: (b, b.reshape(b.shape[0], -1).sum(-1, keepdims=True))
            )(
                np.minimum((imm2 * in1 + s1) * in1 + s0 - in0.astype(np.float32), 0)
            ),
        ),
    )
    _CACHE["ops"] = (absdiff, lin_red, lp_min_red)
    return _CACHE["ops"]


def _register_const(nc, value, dtype=F32):
    t = nc.alloc_sbuf_tensor(f"const-{dtype.name}-{value}", [128, 1], dtype)
    nc.gpsimd.memset(t.ap(), value)
    nc.const_aps.aps[(dtype, value)] = t.ap()


def _build():
    absdiff, lin_red, lp_min_red = _get_ops()
    nc = bacc.Bacc(None, target_bir_lowering=False)
    _register_const(nc, 0.5)
    nc.all_engine_barrier()
    x_ext = nc.declare_dram_parameter("x", [ROWS, COLS], F32, isOutput=False)
    y_ext = nc.declare_dram_parameter("y", [ROWS, COLS], F32, isOutput=False)
    out_ext = nc.declare_dram_parameter("out", [128, 1], F32, isOutput=True)

    with TileContext(nc) as tc:
        with (
            tc.tile_pool(name="io", bufs=3) as iop,
            tc.tile_pool(name="work", bufs=3) as wp,
            tc.tile_pool(name="accp", bufs=1) as accp,
        ):
            accN = accp.tile([128, NT], F32, tag="accN")
            accA = accp.tile([128, NT], F32, tag="accA")
            accB = accp.tile([128, NT], F32, tag="accB")

            for t in range(NT):
                xt = iop.tile([128, COLS], F32, tag="x", name=f"x_{t}")
                yt = iop.tile([128, COLS], F32, tag="y", name=f"y_{t}")
                nc.sync.dma_start(out=xt[:, :], in_=x_ext[t * 128:(t + 1) * 128, :])
                nc.sync.dma_start(out=yt[:, :], in_=y_ext[t * 128:(t + 1) * 128, :])

                pm = wp.tile([128, COLS], BF16, tag="pm", name=f"pm_{t}")
                nc.gpsimd.tensor_scalar(pm[:, :], yt[:, :], -2.1, None, ALU.add)
                ds = wp.tile([128, COLS], F32, tag="ds", name=f"ds_{t}")
                nc.vector._custom_dve(absdiff, out=ds[:, :], in0=xt[:, :], in1=yt[:, :],
                                      s0=0.5, s1=D_EPS)
                ld = wp.tile([128, COLS], BF16, tag="ld", name=f"ld_{t}")
                nc.scalar.activation(ld[:, :], ds[:, :], AF.Ln, bias=0.5)
                w = wp.tile([128, COLS], BF16, tag="w", name=f"w_{t}")
                nc.vector.tensor_tensor(w[:, :], pm[:, :], ld[:, :], ALU.mult)
                e2 = wp.tile([128, COLS], BF16, tag="e2", name=f"e2_{t}")
                nc.scalar.activation(e2[:, :], w[:, :], AF.Exp, scale=-1.0)
                nl = wp.tile([128, COLS], F32, tag="nl", name=f"nl_{t}")
                nc.scalar.activation(nl[:, :], e2[:, :], AF.Ln, bias=1.0,
                                     accum_out=accN[:, t:t + 1])
                junkA = wp.tile([128, COLS], BF16, tag="junkA", name=f"junkA_{t}")
                nc.vector._custom_dve(lin_red, out=junkA[:, :], in0=ds[:, :], in1=pm[:, :],
                                      s0=QS[0], s1=QS[1], imm2=QS[2],
                                      accum_out=accA[:, t:t + 1])
                junkB = wp.tile([128, COLS], BF16, tag="junkB", name=f"junkB_{t}")
                nc.vector._custom_dve(lp_min_red, out=junkB[:, :], in0=nl[:, :], in1=pm[:, :],
                                      s0=QR[0], s1=QR[1], imm2=QR[2],
                                      accum_out=accB[:, t:t + 1])

            rN = accp.tile([128, 1], F32, tag="rN")
            nc.vector.tensor_reduce(rN[:, :], accN[:, :], mybir.AxisListType.X, ALU.add)
            rA = accp.tile([128, 1], F32, tag="rA")
            nc.vector.tensor_reduce(rA[:, :], accA[:, :], mybir.AxisListType.X, ALU.add)
            rB = accp.tile([128, 1], F32, tag="rB")
            nc.vector.tensor_reduce(rB[:, :], accB[:, :], mybir.AxisListType.X, ALU.add)
            s1 = accp.tile([128, 1], F32, tag="s1")
            nc.vector.tensor_tensor(s1[:, :], rN[:, :], rA[:, :], ALU.add)
            s2 = accp.tile([128, 1], F32, tag="s2")
            nc.vector.tensor_tensor(s2[:, :], s1[:, :], rB[:, :], ALU.add)
            nc.sync.dma_start(out=out_ext[:, :], in_=s2[:, :])

    nc.compile()
    return nc


def _get_nc():
    if "nc" not in _CACHE:
        _CACHE["nc"] = _build()
    return _CACHE["nc"]


def kernel(input, target):
    x = np.ascontiguousarray(input, dtype=np.float32).reshape(N_CORES, ROWS, COLS)
    y = np.ascontiguousarray(target, dtype=np.float32).reshape(N_CORES, ROWS, COLS)
    nc = _get_nc()
    in_maps = [{"x": x[i], "y": y[i]} for i in range(N_CORES)]
    res = run_bass_kernel_spmd(nc, in_maps, core_ids=list(range(N_CORES)))
    total = sum(float(res.results[i]["out"].sum()) for i in range(N_CORES))
    return np.float32(14.0 * total)


# revision 11
# speedup vs baseline: 3.4169x; 2.7242x over previous
"""AdaptiveWingLoss on 8 TRN2 NeuronCores.

Math (theta=0.5, eps=1, alpha=2.1, omega=14):
  d  = |x - y|,  p = 2.1 - y,  pm = y - 2.1,  z = ln2*pm
  nl  = log1p(d^p) = Ln(exp(p*ln d) + 1)
  lp  = log1p(e^z),  sigma = 1/(1+e^-z),  ps = p*sigma
  lin = ps*(2d-1) + lp = A'(y)*(d-0.5) + lp,  A' = 2*ps = -2*pm*sigma(z)
  loss/14 = select(d<0.5, nl, lin)

Key identity: d >= 0.5  <=>  nl >= lp (monotone), so the select dissolves:
  sum(loss)/14 = sum(nl) + sum(A' * relu(d-0.5)) + sum(min(lp - nl, 0))

sigma(z) and lp are approximated by quadratics in pm (max rel err ~1e-3,
end-to-end ~3e-6), evaluated inside fused custom DVE ops with accum=ADD.
All ACT work is in the single {ln, exp} table set (no table switches).
Batch dim sharded across 8 cores; each core emits per-partition partial
sums [128,1]; host adds and scales by 14.
"""
import numpy as np

import concourse.bacc as bacc
import concourse.mybir as mybir
import concourse.dve_ops as dops
from concourse.dve_spec import Spec, Src0, Src1, C0, C1, C2, Zero, lower, maxx, minn, relu, _has_src1
from concourse.dve_uop import DveOpSpec
from concourse.tile import TileContext
from concourse.bass_utils import run_bass_kernel_spmd

N_CORES = 8
ROWS, COLS = 1024, 2048  # per-core shard, fp32 elements
NT = ROWS // 128
LN2 = float(np.log(2.0))
D_EPS = 1e-6

# quadratic fits on pm = y - 2.1 in [-2.1, -1.1] (np.polyfit, deg 2)
# sigma(ln2*pm) ~ S0 + S1*pm + S2*pm^2 ; A' = -2*pm*sigma -> coefs *(-2)
QS = (-2 * 0.5118057, -2 * 0.20058717, -2 * 0.02234001)
# log1p(exp(ln2*pm)) ~ R0 + R1*pm + R2*pm^2
QR = (0.6760186, 0.31594962, 0.04476109)

F32 = mybir.dt.float32
BF16 = mybir.dt.bfloat16
AF = mybir.ActivationFunctionType
ALU = mybir.AluOpType

_CACHE = {}


def _make_dve_op(name, spec):
    """Register a custom DVE op at runtime (name -> free opcode row)."""
    existing = {op.name: op for op in dops.OPS}
    if name in existing:
        return existing[name]
    row = dops._CUSTOM_DVE_ROW_BASE + len(dops.OPS)
    tmp = DveOpSpec(name=name, opcode=row, uops=lower(spec, ver="v3"),
                    rd1_en=_has_src1(spec))
    op = dops.DveOp(name, spec, subdim=False, uops_sha={"v3": tmp.sha("v3")})
    dops.OPS.append(op)
    dops._SUB_OPCODE_FOR_NAME[name] = row
    dops.CUSTOM_DVE_SPECS[name] = spec
    return op


def _get_ops():
    if "ops" in _CACHE:
        return _CACHE["ops"]
    # ds = max(|x - y|, eps) - 0.5
    absdiff = _make_dve_op(
        "AWL_ABSDIFF_SHIFT",
        Spec(
            body=maxx(maxx(Src0 - Src1, Src1 - Src0), C1) - C0,
            reference=lambda in0, in1, s0, s1, imm2: (
                np.maximum(np.abs(in0.astype(np.float32) - in1), s1) - s0
            ),
        ),
    )
    # accum += relu(ds) * pm * (q0 + q1*pm + q2*pm^2); ds=Src0, pm=Src1
    lin_red = _make_dve_op(
        "AWL_LIN_REDUCE",
        Spec(
            body=relu(Src0) * (Src1 * ((C2 * Src1 + C1) * Src1 + C0)),
            accum=dops.add,
            accum_init=Zero,
            reference=lambda in0, in1, s0, s1, imm2: (
                lambda b: (b, b.reshape(b.shape[0], -1).sum(-1, keepdims=True))
            )(
                np.maximum(in0.astype(np.float32), 0)
                * (in1 * ((imm2 * in1 + s1) * in1 + s0))
            ),
        ),
    )
    # accum += min((r0 + r1*pm + r2*pm^2) - nl, 0); nl=Src0, pm=Src1
    lp_min_red = _make_dve_op(
        "AWL_LPMIN_REDUCE",
        Spec(
            body=minn(((C2 * Src1 + C1) * Src1 + C0) - Src0, Zero),
            accum=dops.add,
            accum_init=Zero,
            reference=lambda in0, in1, s0, s1, imm2: (
                lambda b: (b, b.reshape(b.shape[0], -1).sum(-1, keepdims=True))
            )(
                np.minimum((imm2 * in1 + s1) * in1 + s0 - in0.astype(np.float32), 0)
            ),
        ),
    )
    _CACHE["ops"] = (absdiff, lin_red, lp_min_red)
    return _CACHE["ops"]


def _register_const(nc, value, dtype=F32):
    t = nc.alloc_sbuf_tensor(f"const-{dtype.name}-{value}", [128, 1], dtype)
    nc.gpsimd.memset(t.ap(), value)
    nc.const_aps.aps[(dtype, value)] = t.ap()


def _pin_act_table():
    """Force every ACTIVATE onto the combined {ln, exp} table so the
    compiler never inserts per-instruction ACT_TABLE_LOAD switches."""
    if _CACHE.get("act_pinned"):
        return
    orig = bacc.get_activation_tables
    keep = "natural_log_exp_and_others"

    def patched(module_arch):
        tables = dict(orig(module_arch))
        return {k: (v if k == keep else set()) for k, v in tables.items()}

    bacc.get_activation_tables = patched
    _CACHE["act_pinned"] = True


def _build():
    absdiff, lin_red, lp_min_red = _get_ops()
    _pin_act_table()
    nc = bacc.Bacc(None, target_bir_lowering=False)
    _register_const(nc, 0.5)
    nc.all_engine_barrier()
    x_ext = nc.declare_dram_parameter("x", [ROWS, COLS], F32, isOutput=False)
    y_ext = nc.declare_dram_parameter("y", [ROWS, COLS], F32, isOutput=False)
    out_ext = nc.declare_dram_parameter("out", [128, 1], F32, isOutput=True)

    with TileContext(nc) as tc:
        with (
            tc.tile_pool(name="io", bufs=3) as iop,
            tc.tile_pool(name="work", bufs=3) as wp,
            tc.tile_pool(name="accp", bufs=1) as accp,
        ):
            accN = accp.tile([128, NT], F32, tag="accN")
            accA = accp.tile([128, NT], F32, tag="accA")
            accB = accp.tile([128, NT], F32, tag="accB")

            for t in range(NT):
                xt = iop.tile([128, COLS], F32, tag="x", name=f"x_{t}")
                yt = iop.tile([128, COLS], F32, tag="y", name=f"y_{t}")
                nc.sync.dma_start(out=xt[:, :], in_=x_ext[t * 128:(t + 1) * 128, :])
                nc.sync.dma_start(out=yt[:, :], in_=y_ext[t * 128:(t + 1) * 128, :])

                pm = wp.tile([128, COLS], BF16, tag="pm", name=f"pm_{t}")
                nc.vector.tensor_scalar(pm[:, :], yt[:, :], -2.1, None, ALU.add)
                ds = wp.tile([128, COLS], F32, tag="ds", name=f"ds_{t}")
                nc.vector._custom_dve(absdiff, out=ds[:, :], in0=xt[:, :], in1=yt[:, :],
                                      s0=0.5, s1=D_EPS)
                ld = wp.tile([128, COLS], BF16, tag="ld", name=f"ld_{t}")
                nc.scalar.activation(ld[:, :], ds[:, :], AF.Ln, bias=0.5)
                w = wp.tile([128, COLS], BF16, tag="w", name=f"w_{t}")
                nc.vector.tensor_tensor(w[:, :], pm[:, :], ld[:, :], ALU.mult)
                e2 = wp.tile([128, COLS], BF16, tag="e2", name=f"e2_{t}")
                nc.scalar.activation(e2[:, :], w[:, :], AF.Exp, scale=-1.0)
                nl = wp.tile([128, COLS], F32, tag="nl", name=f"nl_{t}")
                nc.scalar.activation(nl[:, :], e2[:, :], AF.Ln, bias=1.0,
                                     accum_out=accN[:, t:t + 1])
                junkA = wp.tile([128, COLS], BF16, tag="junkA", name=f"junkA_{t}")
                nc.vector._custom_dve(lin_red, out=junkA[:, :], in0=ds[:, :], in1=pm[:, :],
                                      s0=QS[0], s1=QS[1], imm2=QS[2],
                                      accum_out=accA[:, t:t + 1])
                junkB = wp.tile([128, COLS], BF16, tag="junkB", name=f"junkB_{t}")
                nc.vector._custom_dve(lp_min_red, out=junkB[:, :], in0=nl[:, :], in1=pm[:, :],
                                      s0=QR[0], s1=QR[1], imm2=QR[2],
                                      accum_out=accB[:, t:t + 1])

            rN = accp.tile([128, 1], F32, tag="rN")
            nc.vector.tensor_reduce(rN[:, :], accN[:, :], mybir.AxisListType.X, ALU.add)
            rA = accp.tile([128, 1], F32, tag="rA")
            nc.vector.tensor_reduce(rA[:, :], accA[:, :], mybir.AxisListType.X, ALU.add)
            rB = accp.tile([128, 1], F32, tag="rB")
            nc.vector.tensor_reduce(rB[:, :], accB[:, :], mybir.AxisListType.X, ALU.add)
            s1 = accp.tile([128, 1], F32, tag="s1")
            nc.vector.tensor_tensor(s1[:, :], rN[:, :], rA[:, :], ALU.add)
            s2 = accp.tile([128, 1], F32, tag="s2")
            nc.vector.tensor_tensor(s2[:, :], s1[:, :], rB[:, :], ALU.add)
            nc.sync.dma_start(out=out_ext[:, :], in_=s2[:, :])

    nc.compile()
    return nc


def _get_nc():
    if "nc" not in _CACHE:
        _CACHE["nc"] = _build()
    return _CACHE["nc"]


def kernel(input, target):
    x = np.ascontiguousarray(input, dtype=np.float32).reshape(N_CORES, ROWS, COLS)
    y = np.ascontiguousarray(target, dtype=np.float32).reshape(N_CORES, ROWS, COLS)
    nc = _get_nc()
    in_maps = [{"x": x[i], "y": y[i]} for i in range(N_CORES)]
    res = run_bass_kernel_spmd(nc, in_maps, core_ids=list(range(N_CORES)))
    total = sum(float(res.results[i]["out"].sum()) for i in range(N_CORES))
    return np.float32(14.0 * total)
